# revision 1
# baseline (speedup 1.0000x reference)
"""Cost volume (tfa CorrelationCost, kernel_size=1, d=4) on 8 TRN2 cores.

out[b, k, y, x] = (1/C) * sum_c prv[b,c,y,x] * nxt_pad[b,c,y+dy,x+dx],
k = dy*9+dx, dy/dx in 0..8, nxt zero-padded by 4 on each spatial side.

Sharding: core i -> (batch b = i//2, H-half h = i%2). Each core gets the
full-C feature maps for its 64 rows (prv) and 72 padded rows (nxt).

Per-core algorithm (fp16 banded matmul):
  - pixels are tiled into 8y x 16x = 128 blocks -> lhsT [C=128, 128pix]
  - rhs = the 16y' x 24x' = 384-col window of padded nxt around the tile
  - one matmul per tile: psum[pix, n=(dy',dx')] holds every needed product
    sum; the 81 useful entries for pixel (q,r) are at n = (q+dy)*24+(r+dx)
  - psum is bulk-evacuated (fp32 -> fp16 cast) to SBUF alternating between
    the vector and scalar engines, staged per x-band, DMA'd to DRAM; the
    host extracts the banded entries and fixes the layout.

Walrus requires single-free-dim matmul APs, so the host pre-packs:
  - prv tile-major: [C, xb, yb, 128 pixels] (prv pre-scaled by 2^-4)
  - nxt in overlapping 24-col x-bands: [C, xb, 72 rows, 24] (scaled 2^-3)
so both matmul operands are contiguous runs. (2^-4 * 2^-3 = 1/C.)
"""

import numpy as np

import concourse.bass as bass
import concourse.tile as tile
from concourse import bacc, mybir
from concourse.bass_utils import run_bass_kernel_spmd

# Problem geometry (hardcoded per spec)
B, C, H, W = 4, 128, 128, 256
D = 4
ND = 2 * D + 1            # 9
K = ND * ND               # 81
HH = H // 2               # 64 rows per core
HP = HH + 2 * D           # 72 padded nxt rows per core
WP = W + 2 * D            # 264 padded nxt cols
YB, XB = 8, 16            # pixel tile: 8 rows x 16 cols = 128 partitions
NY, NX = YB + 2 * D, XB + 2 * D   # 16 x 24 = 384 rhs window
NTY, NTX = HH // YB, W // XB      # 8 x 16 tiles
NWIN = NY * NX            # 384
BAND = HP * NX            # 1728 elements per nxt x-band
N_CORES = 8

F16 = mybir.dt.float16
F32 = mybir.dt.float32


def build_nc():
    nc = bacc.Bacc("TRN2")
    prv_d = nc.declare_dram_parameter("prv_s", [C, NTX * NTY * 128], F16, isOutput=False)
    nxt_d = nc.declare_dram_parameter("nxt_s", [C, NTX * BAND], F16, isOutput=False)
    out_d = nc.declare_dram_parameter(
        "out_g", [NTX, 128, NTY * NWIN], F16, isOutput=True
    )

    with tile.TileContext(nc) as tc:
        with (
            tc.tile_pool(name="inp", bufs=1) as inp,
            tc.tile_pool(name="band", bufs=3) as bp,
            tc.tile_pool(name="psum", bufs=6, space="PSUM") as pp,
            tc.tile_pool(name="stage", bufs=1) as sp,
        ):
            # Engine instructions on TRN2 fit only ONE sync wait; SWDGE
            # (gpsimd) DMAs tolerate two. Structure so every PE/DVE/ACT op
            # and every HWDGE DMA has at most one cross-proc dependency:
            #  - prv: one big HWDGE DMA, no reuse
            #  - nxt: per-band tiles (pool reuse -> WAR wait) via gpsimd
            #  - stage: ONE region, never reused -> evacs only wait on PE
            #  - out: gpsimd DMAs {evac sem, queue sem}
            prv_sb = inp.tile([C, NTX * NTY * 128], F16)
            stage = sp.tile([128, NTX * NTY * NWIN], F16)
            nc.sync.dma_start(prv_sb[:, :], prv_d[:, :])

            for xb in range(NTX):
                band_t = bp.tile([C, BAND], F16, name=f"band_{xb}", tag="band")
                bs = xb * BAND
                nc.gpsimd.dma_start(band_t[:, :], nxt_d[:, bs : bs + BAND])
                # Dummy weight load touching the nxt band: absorbs the
                # band-DMA wait onto a cheap PE instruction so each matmul
                # only carries its psum-slot release wait.
                nc.tensor.ldweights(band_t[:, :1])
                for yb in range(NTY):
                    ps = pp.tile([128, NWIN], F32)
                    lo = (xb * NTY + yb) * 128
                    lhsT = prv_sb[:, lo : lo + 128]
                    rhs = band_t[:, yb * YB * NX : yb * YB * NX + NWIN]
                    nc.tensor.matmul(ps, lhsT, rhs, start=True, stop=True)
                    dst = stage[:, (xb * NTY + yb) * NWIN : (xb * NTY + yb + 1) * NWIN]
                    # One evac engine per x-band so the out-DMA below waits
                    # on a single semaphore; bands alternate engines.
                    if xb % 2 == 0:
                        nc.vector.tensor_copy(dst, ps)
                    else:
                        nc.scalar.copy(dst, ps)
                nc.gpsimd.dma_start(
                    out_d[xb], stage[:, xb * NTY * NWIN : (xb + 1) * NTY * NWIN]
                )
    return nc


def make_in_maps(prv: np.ndarray, nxt: np.ndarray) -> list[dict[str, np.ndarray]]:
    prv = np.asarray(prv, dtype=np.float32)
    nxt = np.asarray(nxt, dtype=np.float32)
    nxt_pad = np.zeros((B, C, H + 2 * D, W + 2 * D), np.float32)
    nxt_pad[:, :, D : D + H, D : D + W] = nxt * np.float32(0.125)
    prv_s = prv * np.float32(0.0625)
    in_maps = []
    for core in range(N_CORES):
        b, h = divmod(core, 2)
        # prv tile-major: [C, xb, yb, q, r]
        p = prv_s[b, :, h * HH : (h + 1) * HH, :].reshape(C, NTY, YB, NTX, XB)
        p = np.ascontiguousarray(p.transpose(0, 3, 1, 2, 4)).reshape(C, -1)
        # nxt x-bands of 24 cols at stride 16: [C, xb, row, 24]
        x = nxt_pad[b, :, h * HH : h * HH + HP, :]          # [C, 72, 264]
        xw = np.lib.stride_tricks.sliding_window_view(x, NX, axis=2)  # [C,72,241,24]
        xw = np.ascontiguousarray(xw[:, :, ::XB, :].transpose(0, 2, 1, 3))  # [C,16,72,24]
        in_maps.append(
            {
                "prv_s": p.astype(np.float16),
                "nxt_s": xw.reshape(C, -1).astype(np.float16),
            }
        )
    return in_maps


def _band_index() -> np.ndarray:
    """I[k, m]: column of psum row m holding displacement k's value."""
    dy, dx = np.divmod(np.arange(K), ND)          # [81]
    q, r = np.divmod(np.arange(128), XB)          # [128]
    return (q[None, :] + dy[:, None]) * NX + (r[None, :] + dx[:, None])


def extract_core(G: np.ndarray) -> np.ndarray:
    """[NTX, 128, NTY*NWIN] garbage dump -> [K, HH, W] fp32."""
    G = np.asarray(G).astype(np.float32)
    G = G.reshape(NTX, 128, NTY, NWIN).transpose(2, 0, 1, 3)  # [yb, xb, m, n]
    I = _band_index()
    M = np.arange(128)[None, :]
    r = G[:, :, M, I]                                   # [yb, xb, k, m]
    r = r.transpose(2, 0, 1, 3).reshape(K, NTY, NTX, YB, XB)
    return r.transpose(0, 1, 3, 2, 4).reshape(K, HH, W)


def run(prv: np.ndarray, nxt: np.ndarray, trace: bool = False):
    nc = build_nc()
    nc.finalize()
    in_maps = make_in_maps(prv, nxt)
    res = run_bass_kernel_spmd(nc, in_maps, list(range(N_CORES)), trace=trace)
    out = np.empty((B, K, H, W), np.float32)
    for core in range(N_CORES):
        b, h = divmod(core, 2)
        out[b, :, h * HH : (h + 1) * HH, :] = extract_core(
            res.results[core]["out_g"]
        )
    return out, res


def kernel(prv: np.ndarray, nxt: np.ndarray) -> np.ndarray:
    out, _ = run(prv, nxt, trace=False)
    return out


if __name__ == "__main__":
    rng = np.random.default_rng(0)
    prv = rng.standard_normal((B, C, H, W), dtype=np.float32)
    nxt = rng.standard_normal((B, C, H, W), dtype=np.float32)
    out = kernel(prv, nxt)
    print(out.shape, out.dtype)



# revision 2
# speedup vs baseline: 1.1337x; 1.1337x over previous
"""Cost volume (tfa CorrelationCost, kernel_size=1, d=4) on 8 TRN2 cores.

out[b, k, y, x] = (1/C) * sum_c prv[b,c,y,x] * nxt_pad[b,c,y+dy,x+dx],
k = dy*9+dx, dy/dx in 0..8, nxt zero-padded by 4 on each spatial side.

Sharding: core i -> (batch b = i//2, H-half h = i%2). Each core gets the
full-C feature maps for its 64 rows (prv) and 72 padded rows (nxt).

Per-core algorithm (fp16 banded matmul), v2 — HBM-traffic-minimized:
  - pixels are tiled into 8y x 16x = 128 blocks -> lhsT [C=128, 128pix]
  - rhs = the 16y' x 24x' window of UNBANDED padded nxt ([C, 72, 264] in
    SBUF), read via a 2-free-dim access pattern -> no x-band duplication
  - one matmul per tile: psum[pix, n=(dy',dx')]; the 81 useful entries for
    pixel (q,r) are at n = (q+dy)*24 + (r+dx)
  - psum is evacuated (fp32 -> fp16) to a full-384-col SBUF stage,
    alternating vector/scalar engines per y-band
  - out-DMA gathers only the per-q 216-col slabs [24q, 24q+216) that hold
    all 81 useful columns for pixel-row q: 7.08 MB instead of the 12.6 MB
    full dump; the host extracts the banded entries from the slabs.

Traffic per core: prv 4.19 MB + nxt 4.86 MB + out 7.08 MB = 16.1 MB
(vs 23.9 MB for the banded/full-dump v1).

Engine plan: inputs on gpsimd SWDGE (9 nxt row-chunks + 8 prv y-band
chunks, pipelined so matmuls start ~4us in); out slab DMAs on the two
HWDGE rings (SP for vector-evac'd bands, ACT for scalar-evac'd bands,
emitted after that band's evacs so the FIFO never stalls an evac);
dummy ldweights absorb input-DMA waits so each matmul carries only its
psum-slot release wait.
"""

import numpy as np

import concourse.bass as bass
import concourse.tile as tile
from concourse import bacc, mybir
from concourse.bass_utils import run_bass_kernel_spmd

# Problem geometry (hardcoded per spec)
B, C, H, W = 4, 128, 128, 256
D = 4
ND = 2 * D + 1            # 9
K = ND * ND               # 81
HH = H // 2               # 64 rows per core
HP = HH + 2 * D           # 72 padded nxt rows per core
WP = W + 2 * D            # 264 padded nxt cols
YB, XB = 8, 16            # pixel tile: 8 rows x 16 cols = 128 partitions
NY, NX = YB + 2 * D, XB + 2 * D   # 16 x 24 window
NTY, NTX = HH // YB, W // XB      # 8 y-bands x 16 x-tiles
NWIN = NY * NX            # 384
SLAB = ND * NX            # 216 cols per q-slab
N_CORES = 8

F16 = mybir.dt.float16
F32 = mybir.dt.float32


def build_nc():
    nc = bacc.Bacc("TRN2")
    prv_d = nc.declare_dram_parameter("prv_s", [C, NTY * NTX * 128], F16, isOutput=False)
    nxt_d = nc.declare_dram_parameter("nxt_s", [C, HP * WP], F16, isOutput=False)
    out_d = nc.declare_dram_parameter(
        "out_g", [NTY, YB, 16 * NTX * SLAB], F16, isOutput=True
    )

    with tile.TileContext(nc) as tc:
        with (
            tc.tile_pool(name="inp", bufs=1) as inp,
            tc.tile_pool(name="psum", bufs=6, space="PSUM") as pp,
            tc.tile_pool(name="stage", bufs=1) as sp,
        ):
            prv_sb = inp.tile([C, NTY * NTX * 128], F16)
            nxt_sb = inp.tile([C, HP, WP], F16)
            stage = sp.tile([128, NTY * NTX, NWIN], F16)

            # Inputs via SWDGE, interleaved so band yb's deps land early:
            # band yb needs nxt rows [8yb, 8yb+16) (chunks yb, yb+1) and
            # prv chunk yb.
            def nxt_chunk(j):
                nc.gpsimd.dma_start(
                    nxt_sb[:, 8 * j : 8 * j + 8, :],
                    nxt_d[:, 8 * j * WP : (8 * j + 8) * WP],
                )

            def prv_chunk(j):
                lo = j * NTX * 128
                nc.gpsimd.dma_start(
                    prv_sb[:, lo : lo + NTX * 128], prv_d[:, lo : lo + NTX * 128]
                )

            nxt_chunk(0)
            nxt_chunk(1)
            prv_chunk(0)
            for j in range(1, NTY):
                nxt_chunk(j + 1)
                prv_chunk(j)

            for yb in range(NTY):
                # Absorb the input-DMA waits on cheap PE instructions so
                # each matmul below carries only its psum-release wait.
                nc.tensor.ldweights(prv_sb[:, yb * NTX * 128 : yb * NTX * 128 + 1])
                nc.tensor.ldweights(nxt_sb[:, 8 * yb, :1])
                nc.tensor.ldweights(nxt_sb[:, 8 * yb + 8, :1])
                for xb in range(NTX):
                    t = yb * NTX + xb
                    ps = pp.tile([128, NWIN], F32)
                    lhsT = prv_sb[:, t * 128 : (t + 1) * 128]
                    rhs = nxt_sb[:, yb * YB : yb * YB + NY, xb * XB : xb * XB + NX]
                    nc.tensor.matmul(ps, lhsT, rhs, start=True, stop=True)
                    dst = stage[:, t, :]
                    # One evac engine per y-band -> each out-DMA below
                    # waits on a single semaphore; bands alternate engines.
                    if yb % 2 == 0:
                        nc.vector.tensor_copy(dst, ps)
                    else:
                        nc.scalar.copy(dst, ps)
                # Slab gather: partitions 16q..16q+16 only ever need psum
                # cols [24q, 24q+216). Even bands (vector-evac'd) go out on
                # the SP HWDGE ring, odd (scalar-evac'd) on the ACT ring --
                # ACT's FIFO wait is then always already satisfied.
                eng = nc.sync if yb % 2 == 0 else nc.scalar
                for q in range(YB):
                    src = stage[
                        16 * q : 16 * q + 16,
                        yb * NTX : (yb + 1) * NTX,
                        24 * q : 24 * q + SLAB,
                    ]
                    eng.dma_start(out_d[yb, q], src)
    return nc


def make_in_maps(prv: np.ndarray, nxt: np.ndarray) -> list[dict[str, np.ndarray]]:
    prv = np.asarray(prv, dtype=np.float32)
    nxt = np.asarray(nxt, dtype=np.float32)
    nxt_pad = np.zeros((B, C, H + 2 * D, W + 2 * D), np.float32)
    nxt_pad[:, :, D : D + H, D : D + W] = nxt * np.float32(0.125)
    prv_s = prv * np.float32(0.0625)  # 2^-4 * 2^-3 = 1/C
    in_maps = []
    for core in range(N_CORES):
        b, h = divmod(core, 2)
        # prv tile-major, yb-outer: [C, yb, xb, q, r]
        p = prv_s[b, :, h * HH : (h + 1) * HH, :].reshape(C, NTY, YB, NTX, XB)
        p = np.ascontiguousarray(p.transpose(0, 1, 3, 2, 4)).reshape(C, -1)
        # nxt unbanded: [C, 72, 264]
        x = nxt_pad[b, :, h * HH : h * HH + HP, :]
        in_maps.append(
            {
                "prv_s": p.astype(np.float16),
                "nxt_s": np.ascontiguousarray(x).reshape(C, -1).astype(np.float16),
            }
        )
    return in_maps


def extract_core(G: np.ndarray) -> np.ndarray:
    """[NTY, YB, 16*NTX*216] slab dump -> [K, HH, W] fp32.

    G[yb, q, r, xb, c] holds psum col 24q + c of pixel (q, r) in tile
    (yb, xb); displacement k=(dy,dx) lives at c = 24*dy + r + dx.
    """
    G = np.asarray(G).astype(np.float32).reshape(NTY, YB, 16, NTX, SLAB)
    G = G.transpose(0, 1, 3, 2, 4)                    # [yb, q, xb, r, c]
    dy, dx = np.divmod(np.arange(K), ND)              # [81]
    r = np.arange(XB)
    I2 = 24 * dy[:, None] + r[None, :] + dx[:, None]  # [81, 16]
    T = G[:, :, :, r[None, :], I2]                    # [yb, q, xb, 81, 16]
    T = T.transpose(3, 0, 1, 2, 4)                    # [81, yb, q, xb, r]
    return T.reshape(K, HH, W)


def run(prv: np.ndarray, nxt: np.ndarray, trace: bool = False):
    nc = build_nc()
    nc.finalize()
    in_maps = make_in_maps(prv, nxt)
    res = run_bass_kernel_spmd(nc, in_maps, list(range(N_CORES)), trace=trace)
    out = np.empty((B, K, H, W), np.float32)
    for core in range(N_CORES):
        b, h = divmod(core, 2)
        out[b, :, h * HH : (h + 1) * HH, :] = extract_core(
            res.results[core]["out_g"]
        )
    return out, res


def kernel(prv: np.ndarray, nxt: np.ndarray) -> np.ndarray:
    out, _ = run(prv, nxt, trace=False)
    return out


if __name__ == "__main__":
    rng = np.random.default_rng(0)
    prv = rng.standard_normal((B, C, H, W), dtype=np.float32)
    nxt = rng.standard_normal((B, C, H, W), dtype=np.float32)
    out = kernel(prv, nxt)
    print(out.shape, out.dtype)
